# revision 1
# baseline (speedup 1.0000x reference)
"""Bahdanau-style attention kernel for Trainium2, SPMD across 8 NeuronCores.

Math (per batch row b):
    dec_proj = decoder_state @ W_dec + b_transform            # [D]
    enc_proj = encoder_outputs[b] @ W_enc                     # [S, D]
    feats    = tanh(enc_proj + dec_proj)                      # [S, D]
    scores   = feats @ v_scorer                               # [S]
    probs    = softmax(where(mask, scores, -1e9))             # [S]

Distribution: data-parallel on batch (8 batches per core, weights
replicated), with mask-aware work packing: the mask is length-style, so
positions >= length contribute exactly 0 to the output. Work is split
into units of (batch, 256-column s-chunk) covering only unmasked
columns, and units are bin-packed across the 8 cores (each core gets
exactly 8 batches; a batch's units all stay on its core). The Bass
graph is parametrized only by U (units per core), so graphs are cached
per U and everything else (per-unit decoder columns, one-hot v
selectors, slot maps, masks) is input data.

Device-side formulation (per core, U units):
  - Transposed layout enc_projT[d, s]: lhsT (stationary) = W_enc tiles
    in natural [e, d] layout, moving operand = encT[e, s] (host
    pre-transposes + casts bf16), dec_proj bias rides the partition
    axis (native ScalarE activation bias), v-dot is a K=D matmul.
  - Unit u's scores land on partition u of one PSUM tile [U, 256] via
    one-hot-column v weights; all score matmuls form one accumulation
    group (rows not owned by a matmul receive +0).
  - Softmax without max-subtraction (scores are tanh-bounded):
    probs = exp(s)*mask01 / sum; per-batch sums are formed from
    per-unit partial sums with two tiny matmuls against 0/1 unit<->slot
    maps, so no cross-partition vector ops are needed.
"""

import math

import numpy as np
import ml_dtypes

B, S, E, D = 64, 1024, 1024, 512
N_CORES = 8
BPC = B // N_CORES  # batches per core
NE = E // 128  # 8 e-tiles
ND = D // 128  # 4 d-tiles
NK = D // 128  # 4 k-tiles for W_dec (K = DEC_DIM = 512)
CH = 256  # columns per work unit
NCH = S // CH  # 4 chunk slots per batch

_cache = {}


def _build(U):
    """Build + compile the SPMD graph for U work units per core."""
    from contextlib import ExitStack

    import concourse.bass as bass
    import concourse.tile as tile
    from concourse import bacc, mybir

    f32 = mybir.dt.float32
    bf16 = mybir.dt.bfloat16
    AF = mybir.ActivationFunctionType

    nc = bacc.Bacc(
        "TRN2", target_bir_lowering=False, debug=False, num_devices=N_CORES
    )

    encU = nc.dram_tensor("encU", [U, 128, NE, CH], bf16, kind="ExternalInput").ap()
    wenc = nc.dram_tensor("wenc", [ND, 128, NE, 128], bf16, kind="ExternalInput").ap()
    wdec = nc.dram_tensor("wdec", [128, NK, D], bf16, kind="ExternalInput").ap()
    dectU = nc.dram_tensor("dectU", [128, NK, U], bf16, kind="ExternalInput").ap()
    bt = nc.dram_tensor("bt", [128, ND], f32, kind="ExternalInput").ap()
    # one-hot-v selector via stride trick: vU[:, t, U] = v tile t, zeros
    # elsewhere; the slice [:, t, U-u : 2U-u] is then a [128, U] matrix
    # whose only nonzero column is column u
    vU = nc.dram_tensor("vU", [128, ND, 2 * U], bf16, kind="ExternalInput").ap()
    # additive log-mask (0 valid / -1e9 masked) + identity, folded into the
    # score PSUM via one rank-U matmul so exp's accum_out gives masked sums
    maskl = nc.dram_tensor("maskl", [U, CH], bf16, kind="ExternalInput").ap()
    idU = nc.dram_tensor("idU", [U, U], bf16, kind="ExternalInput").ap()
    u2s = nc.dram_tensor("u2s", [U, BPC], f32, kind="ExternalInput").ap()
    s2u = nc.dram_tensor("s2u", [BPC, U], f32, kind="ExternalInput").ap()
    out = nc.dram_tensor("out", [U, CH], f32, kind="ExternalOutput").ap()

    with tile.TileContext(nc) as tc:
        with ExitStack() as ctx:
            const = ctx.enter_context(tc.tile_pool(name="const", bufs=1))
            enc_pool = ctx.enter_context(tc.tile_pool(name="encp", bufs=5))
            fpool = ctx.enter_context(tc.tile_pool(name="feats", bufs=10))

            # PE warmup: the memset is the very first gpsimd instruction so
            # the dependency-free dummy matmuls can start ASAP, fill the
            # startup DMA wait, and trip the HAM clock-gate to 2.4 GHz
            warm_sb = const.tile([128, 640], bf16)
            nc.gpsimd.memset(warm_sb[:], 0.0)

            # HBM bandwidth is the startup constraint, so everything rides
            # ONE ring in dependency order: dec-phase weights, then the t=0
            # d-tile of W_enc + unit 0 (the first main matmul group's deps),
            # then the rest
            wdec_sb = const.tile([128, NK, D], bf16)
            nc.sync.dma_start(wdec_sb[:], wdec)
            dectU_sb = const.tile([128, NK, U], bf16)
            nc.sync.dma_start(dectU_sb[:], dectU)
            wenc_sb = const.tile([128, ND, NE, 128], bf16)
            ets = []
            npre = min(3, U)
            et0 = enc_pool.tile([128, NE, CH], bf16, tag="et", name="et")
            nc.sync.dma_start(wenc_sb[:, 0, :, :], wenc[0])
            nc.sync.dma_start(et0[:], encU[0])
            ets.append(et0)
            for t in range(1, ND):
                nc.sync.dma_start(wenc_sb[:, t, :, :], wenc[t])
            for u in range(1, npre):
                et = enc_pool.tile([128, NE, CH], bf16, tag="et", name="et")
                nc.sync.dma_start(et[:], encU[u])
                ets.append(et)
            bt_sb = const.tile([128, ND], f32)
            nc.scalar.dma_start(bt_sb[:], bt)
            # late-needed constants go on the sync ring BEHIND the unit
            # prefetches so they don't steal HBM bandwidth at startup
            vU_sb = const.tile([128, ND, 2 * U], bf16)
            nc.sync.dma_start(vU_sb[:], vU)
            maskl_sb = const.tile([U, CH], bf16)
            nc.sync.dma_start(maskl_sb[:], maskl)
            idU_sb = const.tile([U, U], bf16)
            nc.sync.dma_start(idU_sb[:], idU)
            u2s_sb = const.tile([U, BPC], f32)
            nc.sync.dma_start(u2s_sb[:], u2s)
            s2u_sb = const.tile([BPC, U], f32)
            nc.sync.dma_start(s2u_sb[:], s2u)
            decprojU_sb = const.tile([128, ND, U], f32)

            with tc.tile_pool(name="warmp", bufs=1, space="PSUM") as wpool:
                wps = wpool.tile([128, 512], f32, name="wps")
                for _ in range(12):
                    nc.tensor.matmul(
                        wps[:],
                        lhsT=warm_sb[:, 0:128],
                        rhs=warm_sb[:, 128:640],
                        start=True,
                        stop=True,
                        skip_group_check=True,
                    )

            # --- dec_projT[d, u] = W_dec^T @ dec_colsU + b ---
            with tc.tile_pool(name="dpsum", bufs=2, space="PSUM") as dpsum:
                for t in range(ND):
                    ps = dpsum.tile([128, U], f32, name="dps")
                    for k in range(NK):
                        nc.tensor.matmul(
                            ps[:],
                            lhsT=wdec_sb[:, k, bass.ts(t, 128)],
                            rhs=dectU_sb[:, k, :],
                            start=(k == 0),
                            stop=(k == NK - 1),
                        )
                    nc.scalar.add(decprojU_sb[:, t, :], ps[:], bt_sb[:, t : t + 1])

            spsum = ctx.enter_context(tc.tile_pool(name="spsum", bufs=1, space="PSUM"))
            scU = spsum.tile([U, CH], f32, name="scU")
            n_sc_mms = U * ND + 1  # +1 for the log-mask rank-U add
            state = {"count": 0}
            pending = []  # delayed score MMs: (u, t, ft)

            def flush_pending():
                for (u, t, ft) in pending:
                    state["count"] += 1
                    nc.tensor.matmul(
                        scU[:],
                        lhsT=vU_sb[:, t, U - u : 2 * U - u],
                        rhs=ft[:],
                        start=(state["count"] == 1),
                        stop=(state["count"] == n_sc_mms),
                        skip_group_check=True,
                    )
                pending.clear()

            # --- main loop over units ---
            with tc.tile_pool(name="mpsum", bufs=6, space="PSUM") as mpsum:
                for u in range(U):
                    if u < npre:
                        et = ets[u]
                    else:
                        et = enc_pool.tile([128, NE, CH], bf16, tag="et", name="et")
                        nc.sync.dma_start(et[:], encU[u])
                    prev = list(pending)
                    pending.clear()
                    this_unit = []
                    for t in range(ND):
                        ps = mpsum.tile([128, CH], f32, tag="mp", name="mp")
                        for e in range(NE):
                            nc.tensor.matmul(
                                ps[:],
                                lhsT=wenc_sb[:, t, e, :],
                                rhs=et[:, e, :],
                                start=(e == 0),
                                stop=(e == NE - 1),
                            )
                        ft = fpool.tile([128, CH], bf16, tag="ft", name="ft")
                        nc.scalar.activation(
                            ft[:],
                            ps[:],
                            func=AF.Tanh,
                            bias=decprojU_sb[:, t, u : u + 1],
                            scale=1.0,
                        )
                        this_unit.append((u, t, ft))
                    # emit previous unit's score MMs now (their tanh inputs
                    # are ready, so PE doesn't stall on ACT)
                    pending.extend(prev)
                    flush_pending()
                    pending.extend(this_unit)
                flush_pending()
                # fold the additive log-mask into the scores
                state["count"] += 1
                nc.tensor.matmul(
                    scU[:],
                    lhsT=idU_sb[:],
                    rhs=maskl_sb[:],
                    start=False,
                    stop=True,
                    skip_group_check=True,
                )

            # --- masked softmax epilogue in unit space ---
            with tc.tile_pool(name="tpsum", bufs=2, space="PSUM") as tpsum, \
                 tc.tile_pool(name="epi", bufs=1) as epool:
                escU = epool.tile([U, CH], f32, name="escU")
                usums = epool.tile([U, 1], f32, name="usums")
                nc.scalar.activation(
                    escU[:], scU[:], func=AF.Exp, accum_out=usums[:]
                )
                # per-slot (batch) sums: bsums[s] = sum_u u2s[u, s] * usums[u]
                bs_ps = tpsum.tile([BPC, 1], f32, name="bs_ps")
                nc.tensor.matmul(
                    bs_ps[:], lhsT=u2s_sb[:], rhs=usums[:], start=True, stop=True
                )
                brs = epool.tile([BPC, 1], f32, name="brs")
                nc.vector.reciprocal(brs[:], bs_ps[:])
                # broadcast back to units: rbU[u] = sum_s s2u[s, u] * brs[s]
                rb_ps = tpsum.tile([U, 1], f32, name="rb_ps")
                nc.tensor.matmul(
                    rb_ps[:], lhsT=s2u_sb[:], rhs=brs[:], start=True, stop=True
                )
                rbU = epool.tile([U, 1], f32, name="rbU")
                nc.vector.tensor_copy(rbU[:], rb_ps[:])
                # split the final scale + store into column halves on two
                # DMA rings so the first store overlaps the second scale
                probsU = epool.tile([U, CH], f32, name="probsU")
                H = CH // 2
                nc.vector.tensor_scalar_mul(probsU[:, 0:H], escU[:, 0:H], rbU[:])
                nc.sync.dma_start(out[:, 0:H], probsU[:, 0:H])
                nc.vector.tensor_scalar_mul(probsU[:, H:CH], escU[:, H:CH], rbU[:])
                nc.scalar.dma_start(out[:, H:CH], probsU[:, H:CH])

    nc.compile()
    return nc


def _assign(lengths):
    """Bin-pack batches (weight = #units) onto 8 cores, 8 batches each.

    Returns (per_core_batches, U) where per_core_batches[i] is a list of
    8 global batch indices (slot order) and U is the max unit count.
    """
    w = [max(1, math.ceil(l / CH)) for l in lengths]
    order = sorted(range(B), key=lambda b: -w[b])
    loads = [0] * N_CORES
    members = [[] for _ in range(N_CORES)]
    for b in order:
        cands = [i for i in range(N_CORES) if len(members[i]) < BPC]
        i = min(cands, key=lambda i: (loads[i], len(members[i])))
        members[i].append(b)
        loads[i] += w[b]
    U = max(loads)
    return members, U


def _prep_inputs(decoder_state, encoder_outputs, input_mask, W_transform,
                 b_transform, v_scorer, members, U):
    bf16 = ml_dtypes.bfloat16
    W_dec = W_transform[:D]
    W_enc = W_transform[D:]

    # [e, d] -> [t_d, p, t_e, dcol]
    wenc_h = np.ascontiguousarray(
        W_enc.astype(bf16).reshape(NE, 128, ND, 128).transpose(2, 1, 0, 3)
    )
    wdec_h = np.ascontiguousarray(
        W_dec.astype(bf16).reshape(NK, 128, D).transpose(1, 0, 2)
    )
    bt_h = np.ascontiguousarray(b_transform.astype(np.float32).reshape(ND, 128).T)
    v_tiles = v_scorer.astype(np.float32).reshape(ND, 128).T.astype(bf16)  # [128, ND]

    lengths = input_mask.sum(axis=1).astype(int)
    mask_f = input_mask.astype(np.float32)
    enc_bf = encoder_outputs.astype(bf16)  # [B, S, E]
    dec_bf = decoder_state.astype(bf16)  # [B, D]

    in_maps = []
    unit_maps = []  # per core: list of (global_batch, chunk) per unit (None = dummy)
    for core in range(N_CORES):
        batches = members[core]
        units = []
        for slot, gb in enumerate(batches):
            for c in range(max(1, math.ceil(lengths[gb] / CH))):
                units.append((gb, slot, c))
        assert len(units) <= U
        encU_h = np.zeros((U, 128, NE, CH), dtype=bf16)
        dectU_h = np.zeros((128, NK, U), dtype=bf16)
        vU_h = np.zeros((128, ND, 2 * U), dtype=bf16)
        vU_h[:, :, U] = v_tiles
        maskl_h = np.full((U, CH), -1e9, dtype=bf16)
        idU_h = np.eye(U, dtype=bf16)
        u2s_h = np.zeros((U, BPC), dtype=np.float32)
        s2u_h = np.zeros((BPC, U), dtype=np.float32)
        for u, (gb, slot, c) in enumerate(units):
            s0 = c * CH
            s1 = min(s0 + CH, int(lengths[gb]))
            n = s1 - s0
            # [n, E] -> [E, n] -> [NE, 128, n] -> [128, NE, n]
            blk = enc_bf[gb, s0:s1]
            encU_h[u, :, :, :n] = blk.T.reshape(NE, 128, n).transpose(1, 0, 2)
            dectU_h[:, :, u] = dec_bf[gb].reshape(NK, 128).T
            maskl_h[u, :n] = np.where(mask_f[gb, s0:s1] > 0, 0.0, -1e9).astype(bf16)
            u2s_h[u, slot] = 1.0
            s2u_h[slot, u] = 1.0
        in_maps.append(
            {
                "encU": encU_h,
                "wenc": wenc_h,
                "wdec": wdec_h,
                "dectU": np.ascontiguousarray(dectU_h),
                "bt": bt_h,
                "vU": vU_h,
                "maskl": maskl_h,
                "idU": idU_h,
                "u2s": u2s_h,
                "s2u": s2u_h,
            }
        )
        unit_maps.append(units)
    return in_maps, unit_maps


def kernel(decoder_state, encoder_outputs, input_mask, W_transform, b_transform,
           v_scorer, _trace=False):
    from concourse.bass_utils import run_bass_kernel_spmd

    decoder_state = np.asarray(decoder_state)
    encoder_outputs = np.asarray(encoder_outputs)
    input_mask = np.asarray(input_mask)
    W_transform = np.asarray(W_transform)
    b_transform = np.asarray(b_transform)
    v_scorer = np.asarray(v_scorer)

    lengths = input_mask.sum(axis=1).astype(int)
    members, U = _assign(lengths)

    key = ("nc", U)
    if key not in _cache:
        _cache[key] = _build(U)
    nc = _cache[key]

    in_maps, unit_maps = _prep_inputs(
        decoder_state, encoder_outputs, input_mask, W_transform, b_transform,
        v_scorer, members, U
    )
    res = run_bass_kernel_spmd(nc, in_maps, core_ids=list(range(N_CORES)), trace=_trace)

    out_full = np.zeros((B, S), dtype=np.float32)
    for core in range(N_CORES):
        o = res.results[core]["out"]  # [U, CH]
        for u, (gb, slot, c) in enumerate(unit_maps[core]):
            out_full[gb, c * CH : c * CH + CH] = o[u]
    if _trace:
        _cache["last_result"] = res
    return out_full

